# revision 12
# baseline (speedup 1.0000x reference)
# Trainium2 Bass kernel for CustomFullyConnectedLayer:
#   y = x @ W.T,  W[(c+i)%N, c] += V[i, c] for i in diag_pos  (banded weight)
# Strategy: data-parallel over batch across 8 cores; compute y.T directly
# from host-pre-transposed x so no on-chip transposes are needed.
#   y.T[r, b] = sum_i V[i, r-i] * x.T[(r-i)%N, b]
# Per 128-row output tile j (r = 128j+q):
#   psum[q, b] = sum_p A_j[p, q] xT[128j+p, b] + sum_pp B_j[pp, q] xT[128(j-1)+64+pp, b]
# A_j: in-tile band (i <= q); B_j: 64-row wrap band from the previous
# feature tile (i > q), zero-padded to base partition 64. Both host-built.
# The PE HAM clock gate is cold (1.2 GHz) by default and warms to 2.4 GHz
# after ~3.4us of sustained activity; the kernel keeps the matmul stream
# gapless (deep psum pool, big copies, batched stores) so it stays warm.
import os
import sys

import numpy as np

if "/opt/trn_rl_repo" not in sys.path:
    sys.path.insert(0, "/opt/trn_rl_repo")

import ml_dtypes

BATCH = 8192
N = 3072
NCORES = 8
BC = BATCH // NCORES          # 1024 batch cols per core
NJ = N // 128                 # 24 feature/output tiles
HB = BC // 2                  # psum half: one fp32 bank
SG = 4                        # output tiles per store group

_CACHE = {}
LAST_RESULTS = None


def _build_program():
    import concourse.mybir as mybir
    import concourse.tile as tile
    from concourse import bacc

    cdt = mybir.dt.bfloat16
    f32 = mybir.dt.float32

    nc = bacc.Bacc("TRN2", target_bir_lowering=False, debug=False)
    xT = nc.dram_tensor("xT", [N, BC], cdt, kind="ExternalInput")
    wa = nc.dram_tensor("wa", [128, NJ, 128], cdt, kind="ExternalInput")
    wb = nc.dram_tensor("wb", [64, NJ, 128], cdt, kind="ExternalInput")
    yT = nc.dram_tensor("yT", [128, NJ, BC], cdt, kind="ExternalOutput")

    with tile.TileContext(nc) as tc:
        with (
            tc.tile_pool(name="consts", bufs=1) as consts,
            tc.tile_pool(name="yout", bufs=2) as yout,
            tc.tile_pool(name="ps", bufs=4, space="PSUM") as psp,
        ):
            a_sb = consts.tile([128, NJ, 128], cdt)
            b_sb = consts.tile([128, NJ, 128], cdt)  # rows 64:128 hold B
            x_sb = consts.tile([128, NJ, BC], cdt)
            wsrc = consts.tile([128, 128], cdt)

            # Weights first, in chunks, on the vector queue so the first
            # matmuls are never weight-blocked.
            WC = 6
            for g in range(NJ // WC):
                nc.scalar.dma_start(
                    out=a_sb[:, WC * g:WC * (g + 1), :],
                    in_=wa[:, WC * g:WC * (g + 1), :],
                )
            nc.scalar.dma_start(out=b_sb[64:128, :, :], in_=wb[:, :, :])
            # x tile 23 first: j=0 needs it for the wrap band.
            nc.sync.dma_start(
                out=x_sb[:, NJ - 1, :], in_=xT[128 * (NJ - 1):, :]
            )
            for j in range(NJ - 1):
                nc.sync.dma_start(
                    out=x_sb[:, j, :], in_=xT[128 * j: 128 * (j + 1), :]
                )

            # PE warm-up on zeros: ~3.4us of sustained activity lifts the
            # HAM clock gate from 1.2 to 2.4 GHz while the DMAs land.
            nc.vector.memset(wsrc, 0.0)
            wps = psp.tile([128, BC], f32, tag="ps")
            for _ in range(40):
                nc.tensor.matmul(
                    wps[:, :128], lhsT=wsrc, rhs=wsrc, start=True, stop=True
                )

            for j in range(NJ):
                jm1 = (j - 1) % NJ
                ps = psp.tile([128, BC], f32, tag="ps")
                nc.tensor.matmul(
                    ps[:, 0:HB], lhsT=a_sb[:, j, :], rhs=x_sb[:, j, 0:HB],
                    start=True, stop=False,
                )
                nc.tensor.matmul(
                    ps[:, HB:BC], lhsT=a_sb[:, j, :], rhs=x_sb[:, j, HB:BC],
                    start=True, stop=False, skip_group_check=True,
                )
                nc.tensor.matmul(
                    ps[:, 0:HB], lhsT=b_sb[64:128, j, :],
                    rhs=x_sb[64:128, jm1, 0:HB],
                    start=False, stop=True, skip_group_check=True,
                )
                nc.tensor.matmul(
                    ps[:, HB:BC], lhsT=b_sb[64:128, j, :],
                    rhs=x_sb[64:128, jm1, HB:BC],
                    start=False, stop=True, skip_group_check=True,
                )
                u = j % SG
                if u == 0:
                    y_gb = yout.tile([128, SG, BC], cdt)
                # one whole-j copy, alternating engines
                if j % 2 == 0:
                    nc.scalar.copy(out=y_gb[:, u, :], in_=ps)
                else:
                    nc.vector.tensor_copy(out=y_gb[:, u, :], in_=ps)
                if u == SG - 1:
                    g = j // SG
                    if g < NJ // SG - 1:
                        nc.gpsimd.dma_start(
                            out=yT[:, SG * g:SG * (g + 1), :], in_=y_gb
                        )
                    else:
                        # split the last group so the drain overlaps
                        for v in range(SG):
                            nc.gpsimd.dma_start(
                                out=yT[:, SG * g + v, :], in_=y_gb[:, v, :]
                            )

    nc.compile()
    return nc


def _host_prep(x, V, diag_pos):
    bf16 = ml_dtypes.bfloat16
    diag = np.asarray(diag_pos).astype(np.int64) % N
    if diag.size and int(diag.max()) > 29:
        raise ValueError(
            f"band kernel supports diag offsets <= 29, got {int(diag.max())}"
        )
    V32 = np.asarray(V, dtype=np.float32)

    A = np.zeros((NJ, 128, 128), np.float32)
    B = np.zeros((NJ, 64, 128), np.float32)
    jj = np.arange(NJ)[:, None]
    for i in diag:
        i = int(i)
        p = np.arange(128 - i)[None, :]
        A[jj, p, p + i] += V32[i, (128 * jj + p) % N]
        if i > 0:
            pp = np.arange(64 - i, 64)[None, :]
            B[jj, pp, pp + i - 64] += V32[i, (128 * (jj - 1) + 64 + pp) % N]
    wa = np.ascontiguousarray(A.transpose(1, 0, 2)).astype(bf16)
    wb = np.ascontiguousarray(B.transpose(1, 0, 2)).astype(bf16)

    xb = np.ascontiguousarray(np.asarray(x, dtype=np.float32)).astype(bf16)
    xb = xb.view(np.uint16)
    xTs = [
        np.ascontiguousarray(xb[k * BC:(k + 1) * BC, :].T).view(bf16)
        for k in range(NCORES)
    ]
    return xTs, wa, wb


def kernel(x, V, diag_pos):
    global LAST_RESULTS
    from concourse.bass_utils import run_bass_kernel_spmd

    if "prog" not in _CACHE:
        _CACHE["prog"] = _build_program()
    nc = _CACHE["prog"]

    xTs, wa, wb = _host_prep(x, V, diag_pos)
    in_maps = [
        {"xT": xTs[k], "wa": wa, "wb": wb} for k in range(NCORES)
    ]
    res = run_bass_kernel_spmd(nc, in_maps, core_ids=list(range(NCORES)))
    LAST_RESULTS = res
    out = np.empty((BATCH, N), np.float32)
    for k in range(NCORES):
        # yT[q, j, b] = y.T[128j+q, b]  ->  y[b, 128j+q]
        arr = np.asarray(res.results[k]["yT"]).astype(np.float32)
        out[k * BC:(k + 1) * BC, :] = arr.transpose(2, 1, 0).reshape(BC, N)
    return out


# revision 17
# speedup vs baseline: 1.1989x; 1.1989x over previous
# Trainium2 Bass kernel for CustomFullyConnectedLayer:
#   y = x @ W.T,  W[(c+i)%N, c] += V[i, c] for i in diag_pos  (banded weight)
# Strategy: data-parallel over batch across 8 cores; compute y.T directly
# from host-pre-transposed x so no on-chip transposes are needed.
#   y.T[r, b] = sum_i V[i, r-i] * x.T[(r-i)%N, b]
# Per 128-row output tile j (r = 128j+q):
#   psum[q, b] = sum_p A_j[p, q] xT[128j+p, b] + sum_pp B_j[pp, q] xT[128(j-1)+64+pp, b]
# A_j: in-tile band (i <= q); B_j: 64-row wrap band from the previous
# feature tile (i > q), zero-padded to base partition 64. Both host-built.
# The PE HAM clock gate is cold (1.2 GHz) by default and warms to 2.4 GHz
# after ~3.4us of sustained activity; the kernel keeps the matmul stream
# gapless (deep psum pool, big copies, batched stores) so it stays warm.
import os
import sys

import numpy as np

if "/opt/trn_rl_repo" not in sys.path:
    sys.path.insert(0, "/opt/trn_rl_repo")

import ml_dtypes

BATCH = 8192
N = 3072
NCORES = 8
BC = BATCH // NCORES          # 1024 batch cols per core
NJ = N // 128                 # 24 feature/output tiles
HB = BC // 2                  # psum half: one fp32 bank
SG = 4                        # output tiles per store group

_CACHE = {}
LAST_RESULTS = None


def _build_program():
    import concourse.mybir as mybir
    import concourse.tile as tile
    from concourse import bacc

    cdt = mybir.dt.bfloat16
    f32 = mybir.dt.float32

    nc = bacc.Bacc("TRN2", target_bir_lowering=False, debug=False)
    xT = nc.dram_tensor("xT", [N, BC], cdt, kind="ExternalInput")
    wa = nc.dram_tensor("wa", [128, NJ, 128], cdt, kind="ExternalInput")
    wb = nc.dram_tensor("wb", [128, NJ, 128], cdt, kind="ExternalInput")
    yT = nc.dram_tensor("yT", [128, NJ, BC], cdt, kind="ExternalOutput")

    with tile.TileContext(nc) as tc:
        with (
            tc.tile_pool(name="consts", bufs=1) as consts,
            tc.tile_pool(name="yout", bufs=2) as yout,
            tc.tile_pool(name="ps", bufs=4, space="PSUM") as psp,
        ):
            a_sb = consts.tile([128, NJ, 128], cdt)
            # full K=128 wrap blocks (only the last 29 rows are nonzero):
            # K<128 (h64) matmuls never lift the HAM clock gate, so B is
            # zero-padded to full contraction depth instead.
            b_sb = consts.tile([128, NJ, 128], cdt)
            x_sb = consts.tile([128, NJ, BC], cdt)
            wsrc = consts.tile([128, 128], cdt)

            # Weights first, in chunks, on the vector queue so the first
            # matmuls are never weight-blocked.
            WC = 6
            for g in range(NJ // WC):
                nc.scalar.dma_start(
                    out=a_sb[:, WC * g:WC * (g + 1), :],
                    in_=wa[:, WC * g:WC * (g + 1), :],
                )
            nc.scalar.dma_start(out=b_sb, in_=wb[:, :, :])
            # x tile 23 first: j=0 needs it for the wrap band.
            nc.sync.dma_start(
                out=x_sb[:, NJ - 1, :], in_=xT[128 * (NJ - 1):, :]
            )
            for j in range(NJ - 1):
                nc.sync.dma_start(
                    out=x_sb[:, j, :], in_=xT[128 * j: 128 * (j + 1), :]
                )

            # PE warm-up on zeros: ~3.4us of sustained activity lifts the
            # HAM clock gate from 1.2 to 2.4 GHz while the DMAs land.
            nc.vector.memset(wsrc, 0.0)
            wps = psp.tile([128, BC], f32, tag="ps")
            for _ in range(40):
                nc.tensor.matmul(
                    wps[:, :128], lhsT=wsrc, rhs=wsrc, start=True, stop=True
                )

            for j in range(NJ):
                jm1 = (j - 1) % NJ
                ps = psp.tile([128, BC], f32, tag="ps")
                nc.tensor.matmul(
                    ps[:, 0:HB], lhsT=a_sb[:, j, :], rhs=x_sb[:, j, 0:HB],
                    start=True, stop=False,
                )
                nc.tensor.matmul(
                    ps[:, HB:BC], lhsT=a_sb[:, j, :], rhs=x_sb[:, j, HB:BC],
                    start=True, stop=False, skip_group_check=True,
                )
                nc.tensor.matmul(
                    ps[:, 0:HB], lhsT=b_sb[:, j, :],
                    rhs=x_sb[:, jm1, 0:HB],
                    start=False, stop=True, skip_group_check=True,
                )
                nc.tensor.matmul(
                    ps[:, HB:BC], lhsT=b_sb[:, j, :],
                    rhs=x_sb[:, jm1, HB:BC],
                    start=False, stop=True, skip_group_check=True,
                )
                u = j % SG
                if u == 0:
                    y_gb = yout.tile([128, SG, BC], cdt)
                # one whole-j copy, alternating engines
                if j % 2 == 0:
                    nc.scalar.copy(out=y_gb[:, u, :], in_=ps)
                else:
                    nc.vector.tensor_copy(out=y_gb[:, u, :], in_=ps)
                if u == SG - 1:
                    g = j // SG
                    if g < NJ // SG - 1:
                        nc.gpsimd.dma_start(
                            out=yT[:, SG * g:SG * (g + 1), :], in_=y_gb
                        )
                    else:
                        # split the last group so the drain overlaps
                        for v in range(SG):
                            nc.gpsimd.dma_start(
                                out=yT[:, SG * g + v, :], in_=y_gb[:, v, :]
                            )

    nc.compile()
    return nc


def _host_prep(x, V, diag_pos):
    bf16 = ml_dtypes.bfloat16
    diag = np.asarray(diag_pos).astype(np.int64) % N
    if diag.size and int(diag.max()) > 29:
        raise ValueError(
            f"band kernel supports diag offsets <= 29, got {int(diag.max())}"
        )
    V32 = np.asarray(V, dtype=np.float32)

    A = np.zeros((NJ, 128, 128), np.float32)
    B = np.zeros((NJ, 128, 128), np.float32)
    jj = np.arange(NJ)[:, None]
    for i in diag:
        i = int(i)
        p = np.arange(128 - i)[None, :]
        A[jj, p, p + i] += V32[i, (128 * jj + p) % N]
        if i > 0:
            pw = np.arange(128 - i, 128)[None, :]
            B[jj, pw, pw + i - 128] += V32[i, (128 * (jj - 1) + pw) % N]
    wa = np.ascontiguousarray(A.transpose(1, 0, 2)).astype(bf16)
    wb = np.ascontiguousarray(B.transpose(1, 0, 2)).astype(bf16)

    xb = np.ascontiguousarray(np.asarray(x, dtype=np.float32)).astype(bf16)
    xb = xb.view(np.uint16)
    xTs = [
        np.ascontiguousarray(xb[k * BC:(k + 1) * BC, :].T).view(bf16)
        for k in range(NCORES)
    ]
    return xTs, wa, wb


def kernel(x, V, diag_pos):
    global LAST_RESULTS
    from concourse.bass_utils import run_bass_kernel_spmd

    if "prog" not in _CACHE:
        _CACHE["prog"] = _build_program()
    nc = _CACHE["prog"]

    xTs, wa, wb = _host_prep(x, V, diag_pos)
    in_maps = [
        {"xT": xTs[k], "wa": wa, "wb": wb} for k in range(NCORES)
    ]
    res = run_bass_kernel_spmd(nc, in_maps, core_ids=list(range(NCORES)))
    LAST_RESULTS = res
    out = np.empty((BATCH, N), np.float32)
    for k in range(NCORES):
        # yT[q, j, b] = y.T[128j+q, b]  ->  y[b, 128j+q]
        arr = np.asarray(res.results[k]["yT"]).astype(np.float32)
        out[k * BC:(k + 1) * BC, :] = arr.transpose(2, 1, 0).reshape(BC, N)
    return out


# revision 21
# speedup vs baseline: 1.2877x; 1.0741x over previous
# Trainium2 Bass kernel for CustomFullyConnectedLayer:
#   y = x @ W.T,  W[(c+i)%N, c] += V[i, c] for i in diag_pos  (banded weight)
# Strategy: data-parallel over batch across 8 cores; compute y.T directly
# from host-pre-transposed x so no on-chip transposes are needed.
#   y.T[r, b] = sum_i V[i, r-i] * x.T[(r-i)%N, b]
# Per 128-row output tile j (r = 128j+q):
#   psum[q, b] = sum_p A_j[p, q] xT[128j+p, b] + sum_pp B_j[pp, q] xT[128(j-1)+64+pp, b]
# A_j: in-tile band (i <= q); B_j: 64-row wrap band from the previous
# feature tile (i > q), zero-padded to base partition 64. Both host-built.
# The PE HAM clock gate is cold (1.2 GHz) by default and warms to 2.4 GHz
# after ~3.4us of sustained activity; the kernel keeps the matmul stream
# gapless (deep psum pool, big copies, batched stores) so it stays warm.
import os
import sys

import numpy as np

if "/opt/trn_rl_repo" not in sys.path:
    sys.path.insert(0, "/opt/trn_rl_repo")

import ml_dtypes

BATCH = 8192
N = 3072
NCORES = 8
BC = BATCH // NCORES          # 1024 batch cols per core
NJ = N // 128                 # 24 feature/output tiles
HB = BC // 2                  # psum half: one fp32 bank
SG = 4                        # output tiles per store group

_CACHE = {}
LAST_RESULTS = None


def _build_program():
    import concourse.mybir as mybir
    import concourse.tile as tile
    from concourse import bacc

    cdt = mybir.dt.bfloat16
    f32 = mybir.dt.float32

    nc = bacc.Bacc("TRN2", target_bir_lowering=False, debug=False)
    xT = nc.dram_tensor("xT", [128, NJ, BC], cdt, kind="ExternalInput")
    wa = nc.dram_tensor("wa", [128, NJ, 128], cdt, kind="ExternalInput")
    wb = nc.dram_tensor("wb", [128, NJ, 128], cdt, kind="ExternalInput")
    yT = nc.dram_tensor("yT", [128, NJ, BC], cdt, kind="ExternalOutput")

    with tile.TileContext(nc) as tc:
        with (
            tc.tile_pool(name="consts", bufs=1) as consts,
            tc.tile_pool(name="yout", bufs=2) as yout,
            tc.tile_pool(name="ps", bufs=4, space="PSUM") as psp,
        ):
            a_sb = consts.tile([128, NJ, 128], cdt)
            # full K=128 wrap blocks (only the last 29 rows are nonzero):
            # K<128 (h64) matmuls never lift the HAM clock gate, so B is
            # zero-padded to full contraction depth instead.
            b_sb = consts.tile([128, NJ, 128], cdt)
            x_sb = consts.tile([128, NJ, BC], cdt)
            wsrc = consts.tile([128, 128], cdt)

            # Weights first, chunked, on the scalar queue so the first
            # matmuls are never weight-blocked.
            WC = 12
            for g in range(NJ // WC):
                nc.scalar.dma_start(
                    out=a_sb[:, WC * g:WC * (g + 1), :],
                    in_=wa[:, WC * g:WC * (g + 1), :],
                )
            nc.scalar.dma_start(out=b_sb, in_=wb[:, :, :])
            # x tiles in pairs; (22,23) first: j=0 needs tile 23 for the
            # wrap band.
            pairs = [NJ - 2] + list(range(0, NJ - 2, 2))
            for j0 in pairs:
                nc.sync.dma_start(
                    out=x_sb[:, j0:j0 + 2, :], in_=xT[:, j0:j0 + 2, :]
                )

            # PE warm-up on zeros: ~3.4us of sustained activity lifts the
            # HAM clock gate from 1.2 to 2.4 GHz while the DMAs land.
            nc.vector.memset(wsrc, 0.0)
            wps = psp.tile([128, BC], f32, tag="ps")
            for _ in range(32):
                nc.tensor.matmul(
                    wps[:, :128], lhsT=wsrc, rhs=wsrc, start=True, stop=True
                )

            for j in range(NJ):
                jm1 = (j - 1) % NJ
                ps = psp.tile([128, BC], f32, tag="ps")
                nc.tensor.matmul(
                    ps[:, 0:HB], lhsT=a_sb[:, j, :], rhs=x_sb[:, j, 0:HB],
                    start=True, stop=False,
                )
                nc.tensor.matmul(
                    ps[:, HB:BC], lhsT=a_sb[:, j, :], rhs=x_sb[:, j, HB:BC],
                    start=True, stop=False, skip_group_check=True,
                )
                nc.tensor.matmul(
                    ps[:, 0:HB], lhsT=b_sb[:, j, :],
                    rhs=x_sb[:, jm1, 0:HB],
                    start=False, stop=True, skip_group_check=True,
                )
                nc.tensor.matmul(
                    ps[:, HB:BC], lhsT=b_sb[:, j, :],
                    rhs=x_sb[:, jm1, HB:BC],
                    start=False, stop=True, skip_group_check=True,
                )
                u = j % SG
                if u == 0:
                    y_gb = yout.tile([128, SG, BC], cdt)
                # one whole-j copy, alternating engines
                if j % 2 == 0:
                    nc.scalar.copy(out=y_gb[:, u, :], in_=ps)
                else:
                    nc.vector.tensor_copy(out=y_gb[:, u, :], in_=ps)
                if u == SG - 1:
                    g = j // SG
                    if g < NJ // SG - 1:
                        nc.gpsimd.dma_start(
                            out=yT[:, SG * g:SG * (g + 1), :], in_=y_gb
                        )
                    else:
                        # split the last group so the drain overlaps
                        for v in range(SG):
                            nc.gpsimd.dma_start(
                                out=yT[:, SG * g + v, :], in_=y_gb[:, v, :]
                            )

    nc.compile()
    return nc


def _host_prep(x, V, diag_pos):
    bf16 = ml_dtypes.bfloat16
    diag = np.asarray(diag_pos).astype(np.int64) % N
    if diag.size and int(diag.max()) > 29:
        raise ValueError(
            f"band kernel supports diag offsets <= 29, got {int(diag.max())}"
        )
    V32 = np.asarray(V, dtype=np.float32)

    A = np.zeros((NJ, 128, 128), np.float32)
    B = np.zeros((NJ, 128, 128), np.float32)
    jj = np.arange(NJ)[:, None]
    for i in diag:
        i = int(i)
        p = np.arange(128 - i)[None, :]
        A[jj, p, p + i] += V32[i, (128 * jj + p) % N]
        if i > 0:
            pw = np.arange(128 - i, 128)[None, :]
            B[jj, pw, pw + i - 128] += V32[i, (128 * (jj - 1) + pw) % N]
    wa = np.ascontiguousarray(A.transpose(1, 0, 2)).astype(bf16)
    wb = np.ascontiguousarray(B.transpose(1, 0, 2)).astype(bf16)

    xb = np.ascontiguousarray(np.asarray(x, dtype=np.float32)).astype(bf16)
    xb = xb.view(np.uint16)
    # partition-major per core: xT[p, j, b] = x.T[128j+p, b]
    xTs = [
        np.ascontiguousarray(
            xb[k * BC:(k + 1) * BC, :].reshape(BC, NJ, 128).transpose(2, 1, 0)
        ).view(bf16)
        for k in range(NCORES)
    ]
    return xTs, wa, wb


def kernel(x, V, diag_pos):
    global LAST_RESULTS
    from concourse.bass_utils import run_bass_kernel_spmd

    if "prog" not in _CACHE:
        _CACHE["prog"] = _build_program()
    nc = _CACHE["prog"]

    xTs, wa, wb = _host_prep(x, V, diag_pos)
    in_maps = [
        {"xT": xTs[k], "wa": wa, "wb": wb} for k in range(NCORES)
    ]
    res = run_bass_kernel_spmd(nc, in_maps, core_ids=list(range(NCORES)))
    LAST_RESULTS = res
    out = np.empty((BATCH, N), np.float32)
    for k in range(NCORES):
        # yT[q, j, b] = y.T[128j+q, b]  ->  y[b, 128j+q]
        arr = np.asarray(res.results[k]["yT"]).astype(np.float32)
        out[k * BC:(k + 1) * BC, :] = arr.transpose(2, 1, 0).reshape(BC, N)
    return out
